# revision 37
# baseline (speedup 1.0000x reference)
"""Trainium2 Bass kernel for CrossFormerAttention-style GNN message passing.

Reference (N=50000 nodes, E=1600000 edges, 8 heads x 16 dims):
    Qh = (h_add @ WQ).reshape(N, 8, 16)
    Kh = (h @ WK).reshape(N, 8, 16)
    Vh = (h @ WV).reshape(N, 8, 16)
    score = sum(Kh[src] * Qh[dst], -1)             # [E, 8, 1]
    wV = segment_sum(Vh[src] * score, dst, N)      # [N, 8, 16]
    out = wV / N

Sharding: edges partitioned by dst range across 8 cores (6250 nodes/core);
each core owns a disjoint output slice -> no collective.

v2 design vs baseline:
- Phase A: host uploads h TRANSPOSED in bf16 ([128, n_pad]); per 128-chunk
  the chunk is the PE lhsT directly (no PE transpose, no DVE cast), rhs is
  the fused WK|WV [128, 256]; PSUM->SBUF copies alternate ACT/DVE; stores
  batched 8 chunks per DMA.  Q table stays SBUF-resident (1/N folded into
  WQ on host).
- Phase B: per-edge Q gather replaced by a one-hot matmul against the SBUF
  Q-block (onehotT uploaded per subchunk from host as bf16 - no Q7 cost).
  KV gathers batched per 4096-edge window, 2 calls (A: src<32768 subchunks
  first, B rest) to amortize the ~1us SWDGE fixed cost.  Score replication
  runs on the Scalar engine; all other elementwise work on DVE.
"""

from contextlib import ExitStack

import numpy as np
import ml_dtypes

import concourse.bass as bass
import concourse.mybir as mybir
from concourse import bass_utils
from concourse.bacc import Bacc
from concourse.tile import TileContext

P = 128
N_NODES = 50000
N_EDGES = 1600000
IN_DIM = 128
NUM_HEADS = 8
OUT_DIM = 16
N_CORES = 8
NODES_PER_CORE = N_NODES // N_CORES  # 6250
I16_BASE = 32768
GROUP_J = 8          # subchunks per compute group (1024 edges)
WIN_G = 4            # groups per gather window (4096 edges)
WIN_J = GROUP_J * WIN_G

F32 = mybir.dt.float32
BF16 = mybir.dt.bfloat16
F8 = mybir.dt.float8e4
I16 = mybir.dt.int16

ACT_REP = True      # replicate score on Scalar engine (else DVE bcast mult)
GCAP = 8             # max subchunks per gather call (8 -> 1024 idxs, one
                     # 64-descriptor packet per SDMA engine; multi-packet
                     # mode degenerates to one packet per descriptor)
QSEL_SPLIT = False    # per-subchunk bank-aligned Qsel PSUM tiles


def _ceil_to(x, m):
    return ((x + m - 1) // m) * m


def _bf(a):
    return np.asarray(a, dtype=np.float32).astype(ml_dtypes.bfloat16)


def _bf_bits(a):
    """bf16 bit pattern as int16 (for fused i16 uploads, bitcast on device)."""
    return _bf(a).view(np.int16)


def shard_edges(src, dst):
    """Partition edges by dst range; per core sort by dst; per 128-dst-block
    split into A (src<32768) / B subchunks of 128 edges; shared schedule
    (max counts over cores); subchunks regrouped per 32-subchunk window with
    A-subchunks first so each window needs only 2 gather calls."""
    src = np.asarray(src).astype(np.int64)
    dst = np.asarray(dst).astype(np.int64)

    order = np.argsort(dst, kind="stable")
    ds, ss = dst[order], src[order]
    bounds = np.searchsorted(ds, np.arange(N_CORES + 1) * NODES_PER_CORE)
    n_blocks = _ceil_to(NODES_PER_CORE, P) // P  # 49

    # per (core, block) A/B edge lists
    edges = [[None] * n_blocks for _ in range(N_CORES)]
    nA = np.zeros((N_CORES, n_blocks), dtype=np.int64)
    nB = np.zeros((N_CORES, n_blocks), dtype=np.int64)
    for c in range(N_CORES):
        sl = slice(bounds[c], bounds[c + 1])
        loc = ds[sl] - c * NODES_PER_CORE
        sc = ss[sl]
        bs = np.searchsorted(loc // P, np.arange(n_blocks + 1))
        for b in range(n_blocks):
            s2 = slice(bs[b], bs[b + 1])
            l2, s3 = loc[s2], sc[s2]
            am = s3 < I16_BASE
            edges[c][b] = ((s3[am], l2[am]), (s3[~am], l2[~am]))
            nA[c, b] = int(am.sum())
            nB[c, b] = int((~am).sum())

    subA = ((nA.max(axis=0) + P - 1) // P).astype(np.int64)
    subB = ((nB.max(axis=0) + P - 1) // P).astype(np.int64)
    if (subA + subB).sum() == 0:
        subA[0] = 1

    # single-tag windows: all-B windows first (their gathers read only the
    # hi table, which phase A builds first, so they interleave under the
    # lo-table build), then all-A windows
    subsB = [(b, 1) for b in range(n_blocks) for _ in range(int(subB[b]))]
    subsB += [(n_blocks - 1, 1)] * ((-len(subsB)) % WIN_J)   # dummies (rel=-1)
    subsA = [(b, 0) for b in range(n_blocks) for _ in range(int(subA[b]))]
    subsA += [(n_blocks - 1, 0)] * ((-len(subsA)) % WIN_J)
    subs = subsB + subsA
    S = len(subs)
    n_win = S // WIN_J
    n_winB = len(subsB) // WIN_J
    blk_of = [s[0] for s in subs]
    tags = [s[1] for s in subs]
    nA_win = [0] * n_winB + [WIN_J] * (n_win - n_winB)

    # real (non-dummy) subchunk positions per (block, tag), in order
    slot_of = {}       # (block, tag) -> list of subchunk indices
    for i, (b, t) in enumerate(subs):
        slot_of.setdefault((b, t), []).append(i)

    # fill per-core data
    kvidx = np.zeros((N_CORES, S * P), dtype=np.int16)
    rel = np.full((N_CORES, S, P), -1.0, dtype=np.float32)
    for c in range(N_CORES):
        for b in range(n_blocks):
            for t in range(2):
                sa, la = edges[c][b][t]
                slots = slot_of.get((b, t), [])
                base = 0 if t == 0 else I16_BASE
                for k, sidx in enumerate(slots):
                    lo = k * P
                    hi = min(lo + P, len(sa))
                    if hi <= lo:
                        break
                    n = hi - lo
                    kvidx[c, sidx * P : sidx * P + n] = (sa[lo:hi] - base).astype(np.int16)
                    rel[c, sidx, :n] = la[lo:hi] - b * P

    # first/last OCCURRENCE per (block, tag): PSUM accumulation runs
    # separately for the B phase (partial staged to SBUF) and the A phase
    first_seen, last_seen = {}, {}
    for i, (b, t) in enumerate(subs):
        if (b, t) not in first_seen:
            first_seen[(b, t)] = i
        last_seen[(b, t)] = i
    first_of = [first_seen[subs[i]] == i for i in range(S)]
    last_of = [last_seen[subs[i]] == i for i in range(S)]
    has_B = [bool(subB[b] > 0) or b == n_blocks - 1 for b in range(n_blocks)]
    has_A = [bool(subA[b] > 0) or b == n_blocks - 1 for b in range(n_blocks)]
    assert all(has_A), "every block needs at least one A subchunk"

    # gather idx layout: per window [128, WIN_J*8] int16, wrapped in 16
    # partitions (idx i at partition i%16, col i//16) replicated x8
    kvidx_w = kvidx.reshape(N_CORES, n_win, WIN_J * 8, 16).transpose(0, 1, 3, 2)
    kvidx_w = np.ascontiguousarray(np.tile(kvidx_w, (1, 1, 8, 1)))  # [C,W,128,WIN_J*8]

    # rel per subchunk in partition-edge layout [128, S] (edge on partition)
    rel_pe = rel.transpose(0, 2, 1)                       # [C, P, S]
    rel_bits = _bf_bits(rel_pe)                           # int16 view of bf16

    # fused per-window i16 upload: [128, WIN_J*8 idx | WIN_J rel]
    fused = np.zeros((N_CORES, n_win, P, WIN_J * 8 + WIN_J), dtype=np.int16)
    fused[:, :, :, : WIN_J * 8] = kvidx_w
    fused[:, :, :, WIN_J * 8 :] = rel_bits.reshape(N_CORES, P, n_win, WIN_J).transpose(0, 2, 1, 3)

    # onehotT upload (fp8: 0/1 exact), window-contiguous:
    # [n_win, 128(node), WIN_J*128] (ohT_w[w, p, j*128+n] = 1 iff
    # rel[subchunk w*W+j, edge n] == p)
    onehotT = np.zeros((N_CORES, S, P, P), dtype=ml_dtypes.float8_e4m3fn)
    r = rel.astype(np.int64)                               # [C, S, P]
    cc, ssi, ee = np.nonzero(r >= 0)
    onehotT[cc, ssi, r[cc, ssi, ee], ee] = 1.0
    onehotT = np.ascontiguousarray(
        onehotT.reshape(N_CORES, n_win, WIN_J, P, P).transpose(0, 1, 3, 2, 4)
    ).reshape(N_CORES, n_win, P, WIN_J * P)

    return {
        "fused": fused,
        "onehotT": onehotT,
        "tags": tags,
        "blk_of": blk_of,
        "first_of": first_of,
        "last_of": last_of,
        "nA_win": nA_win,
        "has_B": has_B,
        "n_blocks": n_blocks,
        "S": S,
        "n_win": n_win,
        "n_winB": n_winB,
    }


def build_program(*, n_nodes_pad, nodes_core_pad, sched):
    nkv_chunks = n_nodes_pad // P          # 391
    nq_chunks = nodes_core_pad // P        # 49
    n_win = sched["n_win"]
    n_winB = sched["n_winB"]
    blk_of = sched["blk_of"]
    first_of = sched["first_of"]
    last_of = sched["last_of"]
    has_B = sched["has_B"]
    J, W = GROUP_J, WIN_J
    SLAB = 16  # chunks per hT slab load (16*128 cols = 0.5MB)

    nc = Bacc(num_swdge_queues=4)

    hT_d = nc.dram_tensor("hT_d", [P, n_nodes_pad], BF16, kind="ExternalInput")
    hqT_d = nc.dram_tensor("hqT_d", [P, nodes_core_pad], BF16, kind="ExternalInput")
    wkv_d = nc.dram_tensor("wkv_d", [P, 2 * IN_DIM], BF16, kind="ExternalInput")
    wq_d = nc.dram_tensor("wq_d", [P, IN_DIM], BF16, kind="ExternalInput")
    fused_d = nc.dram_tensor("fused_d", [n_win, P, W * 8 + W], I16, kind="ExternalInput")
    ohT_d = nc.dram_tensor(
        "ohT_d", [n_win, P, W * P], F8, kind="ExternalInput"
    )
    wv_out = nc.dram_tensor(
        "wv_out", [sched["n_blocks"] * P, IN_DIM], F32, kind="ExternalOutput"
    )

    iota_np = np.tile(np.arange(P, dtype=np.float32), (P, W))  # [P, W*P]
    iota_d = nc.inline_tensor(_bf(iota_np), name="iota_d")

    with TileContext(nc) as tc:
        stack = ExitStack()
        constp = stack.enter_context(tc.tile_pool(name="const", bufs=1))
        dramp = stack.enter_context(tc.tile_pool(name="dram", bufs=1, space="DRAM"))
        hslabp = stack.enter_context(tc.tile_pool(name="hslab", bufs=2))
        projps = stack.enter_context(tc.tile_pool(name="proj_ps", bufs=1, space="PSUM"))
        kvstagep = stack.enter_context(tc.tile_pool(name="kvstage", bufs=2))
        winp = stack.enter_context(tc.tile_pool(name="win", bufs=3))
        grpp = stack.enter_context(tc.tile_pool(name="grp", bufs=3))
        qselps = stack.enter_context(tc.tile_pool(name="qsel_ps", bufs=2, space="PSUM"))
        wvps = stack.enter_context(tc.tile_pool(name="wv_ps", bufs=2, space="PSUM"))
        outp = stack.enter_context(tc.tile_pool(name="outst", bufs=3))
        wvbp = stack.enter_context(tc.tile_pool(name="wvb", bufs=1))

        iota = constp.tile([P, W * P], BF16)
        nc.sync.dma_start(iota, iota_d[:])
        wkv = constp.tile([P, 2 * IN_DIM], BF16, name="wkv")
        nc.sync.dma_start(wkv, wkv_d[:])
        wq = constp.tile([P, IN_DIM], BF16, name="wq")
        nc.sync.dma_start(wq, wq_d[:])

        kv_lo = dramp.tile([I16_BASE, 2 * IN_DIM], BF16, name="kv_lo_tab")
        kv_hi = dramp.tile([n_nodes_pad - I16_BASE, 2 * IN_DIM], BF16, name="kv_hi_tab")
        q_tab = constp.tile([P, nq_chunks * IN_DIM], BF16, name="q_tab")
        # staged B-phase partial outputs, one [P, IN_DIM] f32 slab per block
        wvB = wvbp.tile([P, sched["n_blocks"] * IN_DIM], F32, name="wvB")

        wv_tiles = {}

        def emit_kv_slab(s0, s1, table, base_chunk):
            slab = hslabp.tile([P, SLAB * P], BF16, name="slab")
            nc.sync.dma_start(slab[:, : (s1 - s0) * P], hT_d[:, s0 * P : s1 * P])
            stage = kvstagep.tile([P, SLAB * 2 * IN_DIM], BF16, name="kvstage")
            for ci in range(s0, s1):
                k = ci - s0
                ps = projps.tile([P, 2 * IN_DIM], F32, name="ps_kv")
                nc.tensor.matmul(
                    ps, lhsT=slab[:, k * P : (k + 1) * P], rhs=wkv,
                    start=True, stop=True,
                )
                dst = stage[:, k * 2 * IN_DIM : (k + 1) * 2 * IN_DIM]
                if ci % 2 == 0:
                    nc.scalar.copy(dst, ps)
                else:
                    nc.vector.tensor_copy(dst, ps)
            nc.sync.dma_start(
                table[(s0 - base_chunk) * P : (s1 - base_chunk) * P, :].rearrange(
                    "(c p) f -> p c f", p=P
                ),
                stage.rearrange("p (c f) -> p c f", f=2 * IN_DIM)[:, : s1 - s0, :],
            )

        gq = [0]  # rotating SWDGE queue counter

        def emit_window(w):
            fused_t = winp.tile([P, W * 8 + W], I16, name="fused_t")
            nc.sync.dma_start(fused_t, fused_d[w])
            kv_rows = winp.tile([P, W * 2 * IN_DIM], BF16, name="kv_rows")
            is_B = w < n_winB
            for jk in range(0, W, GCAP):
                j0, j1 = jk, min(jk + GCAP, W)
                nidx = (j1 - j0) * P
                nc.gpsimd.dma_gather(
                    out_ap=kv_rows[
                        :, j0 * 2 * IN_DIM : j1 * 2 * IN_DIM
                    ].rearrange("p (c f) -> p c f", f=2 * IN_DIM),
                    in_ap=kv_hi if is_B else kv_lo,
                    idxs_ap=fused_t[:, j0 * 8 : j1 * 8],
                    num_idxs=nidx,
                    num_idxs_reg=nidx,
                    elem_size=2 * IN_DIM,
                    single_packet=(nidx <= 1024),
                    queue_num=gq[0] % 4,
                )
                gq[0] += 1
            rel_all = fused_t[:, W * 8 :].bitcast(BF16)  # [P, W] bf16

            # scatter one-hot [edge, (j, node)] for the whole window
            onehot_w = winp.tile([P, W * P], BF16, name="onehot_w")
            nc.vector.tensor_tensor(
                out=onehot_w.rearrange("p (j n) -> p j n", j=W),
                in0=iota.rearrange("p (j n) -> p j n", j=W),
                in1=rel_all.unsqueeze(-1).to_broadcast([P, W, P]),
                op=mybir.AluOpType.is_equal,
            )
            # Q selection one-hot (transposed, fp8): one window-sized load
            # on the scalar engine's HWDGE ring (parallel to the SP ring)
            ohT_w = winp.tile([P, W * P], F8, name="ohT_w")
            nc.scalar.dma_start(ohT_w, ohT_d[w])

            for g in range(WIN_G):
                sc0 = w * W + g * J    # first subchunk index of group
                kvg = kv_rows[:, g * J * 2 * IN_DIM : (g + 1) * J * 2 * IN_DIM]
                kv3 = kvg.rearrange("p (j f) -> p j f", f=2 * IN_DIM)
                k3 = kv3[:, :, 0:IN_DIM]
                v3 = kv3[:, :, IN_DIM : 2 * IN_DIM]
                onehot = onehot_w[:, g * J * P : (g + 1) * J * P]
                ohT = ohT_w[:, g * J * P : (g + 1) * J * P]
                # Qsel[e, f] per subchunk via PE, then kq = K * Qsel
                kq = grpp.tile([P, J * IN_DIM], BF16, name="kq")
                qsel_ps = qselps.tile([P, J * IN_DIM], F32, name="qsel_ps")
                for j in range(J):
                    b = blk_of[sc0 + j]
                    nc.tensor.matmul(
                        qsel_ps[:, j * IN_DIM : (j + 1) * IN_DIM],
                        lhsT=ohT[:, j * P : (j + 1) * P],
                        rhs=q_tab[:, b * IN_DIM : (b + 1) * IN_DIM],
                        start=True, stop=True,
                    )
                qsel = grpp.tile([P, J * IN_DIM], BF16, name="qsel")
                nc.scalar.copy(qsel, qsel_ps)
                nc.vector.tensor_tensor(
                    out=kq.rearrange("p (j f) -> p j f", f=IN_DIM),
                    in0=k3,
                    in1=qsel.rearrange("p (j f) -> p j f", f=IN_DIM),
                    op=mybir.AluOpType.mult,
                )
                # score[e, (j,h)] = reduce_d kq (bf16 accum: 16-term dot)
                score = grpp.tile([P, J * NUM_HEADS], BF16, name="score")
                with nc.allow_low_precision("16-term bf16 dot product"):
                    nc.vector.tensor_reduce(
                        out=score,
                        in_=kq.rearrange("p (jh d) -> p jh d", d=OUT_DIM),
                        axis=mybir.AxisListType.X,
                        op=mybir.AluOpType.add,
                    )
                msg = grpp.tile([P, J * IN_DIM], BF16, name="msg")
                score_rep = grpp.tile([P, J * IN_DIM], BF16, name="score_rep")
                nc.scalar.copy(
                    score_rep.rearrange("p (jh d) -> p jh d", d=OUT_DIM),
                    score.unsqueeze(-1).to_broadcast([P, J * NUM_HEADS, OUT_DIM]),
                )
                nc.vector.tensor_tensor(
                    out=msg.rearrange("p (j f) -> p j f", f=IN_DIM),
                    in0=v3,
                    in1=score_rep.rearrange("p (j f) -> p j f", f=IN_DIM),
                    op=mybir.AluOpType.mult,
                )
                # scatter-accumulate into block tiles
                for j in range(J):
                    sc = sc0 + j
                    b = blk_of[sc]
                    if first_of[sc]:
                        wv_tiles[b] = wvps.tile([P, IN_DIM], F32, name="wv_tile")
                    nc.tensor.matmul(
                        wv_tiles[b],
                        lhsT=onehot[:, j * P : (j + 1) * P],
                        rhs=msg[:, j * IN_DIM : (j + 1) * IN_DIM],
                        start=first_of[sc],
                        stop=last_of[sc],
                    )
                    if last_of[sc]:
                        tile = wv_tiles.pop(b)
                        if is_B:
                            # stage the B-phase partial to SBUF
                            nc.scalar.copy(
                                wvB[:, b * IN_DIM : (b + 1) * IN_DIM], tile
                            )
                        else:
                            stage = outp.tile([P, IN_DIM], F32, name="stage")
                            if has_B[b]:
                                nc.vector.tensor_tensor(
                                    out=stage,
                                    in0=tile,
                                    in1=wvB[:, b * IN_DIM : (b + 1) * IN_DIM],
                                    op=mybir.AluOpType.add,
                                )
                            else:
                                nc.scalar.copy(stage, tile)
                            nc.sync.dma_start(
                                wv_out[b * P : (b + 1) * P, :], stage
                            )

        # ---- emission order ----
        # 1) Q projections (q_tab ready before any window compute)
        hq_slab = hslabp.tile([P, nq_chunks * P], BF16, name="hq_slab")
        nc.sync.dma_start(hq_slab, hqT_d[:, : nq_chunks * P])
        for ci in range(nq_chunks):
            ps = projps.tile([P, IN_DIM], F32, name="ps_q")
            nc.tensor.matmul(
                ps, lhsT=hq_slab[:, ci * P : (ci + 1) * P], rhs=wq,
                start=True, stop=True,
            )
            if ci % 2 == 0:
                nc.scalar.copy(q_tab[:, ci * IN_DIM : (ci + 1) * IN_DIM], ps)
            else:
                nc.vector.tensor_copy(q_tab[:, ci * IN_DIM : (ci + 1) * IN_DIM], ps)

        # 2) hi table (chunks I16_BASE//P .. nkv_chunks)
        hi0 = I16_BASE // P
        for s0 in range(hi0, nkv_chunks, SLAB):
            emit_kv_slab(s0, min(s0 + SLAB, nkv_chunks), kv_hi, hi0)

        # 3) lo table interleaved with the all-B windows (their gathers only
        #    read the hi table, so they execute under the lo-table build)
        lo_slabs = [(s0, min(s0 + SLAB, hi0)) for s0 in range(0, hi0, SLAB)]
        b_wins = list(range(n_winB))
        while lo_slabs or b_wins:
            if lo_slabs:
                s0, s1 = lo_slabs.pop(0)
                emit_kv_slab(s0, s1, kv_lo, 0)
            if b_wins:
                emit_window(b_wins.pop(0))
            if b_wins and len(b_wins) > len(lo_slabs):
                emit_window(b_wins.pop(0))

        # 4) all-A windows
        for w in range(n_winB, n_win):
            emit_window(w)
        stack.close()

    nc.finalize()
    return nc


def _make_in_maps(h, h_add, WQ, WK, WV, sched, n_nodes_pad, nodes_core_pad):
    h = np.asarray(h, dtype=np.float32)
    h_add = np.asarray(h_add, dtype=np.float32)
    hT = np.zeros((P, n_nodes_pad), dtype=ml_dtypes.bfloat16)
    hT[:, :N_NODES] = _bf(h.T)
    wkv = np.concatenate(
        [np.asarray(WK, np.float32), np.asarray(WV, np.float32)], axis=1
    )
    wkv = _bf(wkv)
    wq = _bf(np.asarray(WQ, np.float32) / float(N_NODES))
    in_maps = []
    for c in range(N_CORES):
        hqT = np.zeros((P, nodes_core_pad), dtype=ml_dtypes.bfloat16)
        hqT[:, :NODES_PER_CORE] = _bf(
            h_add[c * NODES_PER_CORE : (c + 1) * NODES_PER_CORE].T
        )
        in_maps.append(
            {
                "hT_d": hT,
                "hqT_d": hqT,
                "wkv_d": wkv,
                "wq_d": wq,
                "fused_d": sched["fused"][c],
                "ohT_d": sched["onehotT"][c],
            }
        )
    return in_maps


_TRACE = {"trace": False, "last": None, "tmpdir": None}


def kernel(h, h_add, src, dst, WQ, WK, WV):
    sched = shard_edges(src, dst)
    n_nodes_pad = _ceil_to(N_NODES, P)
    nodes_core_pad = _ceil_to(NODES_PER_CORE, P)

    nc = build_program(
        n_nodes_pad=n_nodes_pad, nodes_core_pad=nodes_core_pad, sched=sched
    )
    in_maps = _make_in_maps(h, h_add, WQ, WK, WV, sched, n_nodes_pad, nodes_core_pad)

    res = bass_utils.run_bass_kernel_spmd(
        nc,
        in_maps,
        core_ids=list(range(N_CORES)),
        trace=_TRACE["trace"],
        tmpdir=_TRACE["tmpdir"],
    )
    _TRACE["last"] = res

    out = np.concatenate(
        [np.asarray(res.results[c]["wv_out"])[:NODES_PER_CORE] for c in range(N_CORES)],
        axis=0,
    )
    return out.reshape(N_NODES, NUM_HEADS, OUT_DIM).astype(np.float32)



# revision 39
# speedup vs baseline: 1.3335x; 1.3335x over previous
"""Trainium2 Bass kernel for CrossFormerAttention-style GNN message passing.

Reference (N=50000 nodes, E=1600000 edges, 8 heads x 16 dims):
    Qh = (h_add @ WQ).reshape(N, 8, 16)
    Kh = (h @ WK).reshape(N, 8, 16)
    Vh = (h @ WV).reshape(N, 8, 16)
    score = sum(Kh[src] * Qh[dst], -1)             # [E, 8, 1]
    wV = segment_sum(Vh[src] * score, dst, N)      # [N, 8, 16]
    out = wV / N

Sharding: edges partitioned by dst range across 8 cores (6250 nodes/core);
each core owns a disjoint output slice -> no collective.

v2 design vs baseline:
- Phase A: host uploads h TRANSPOSED in bf16 ([128, n_pad]); per 128-chunk
  the chunk is the PE lhsT directly (no PE transpose, no DVE cast), rhs is
  the fused WK|WV [128, 256]; PSUM->SBUF copies alternate ACT/DVE; stores
  batched 8 chunks per DMA.  Q table stays SBUF-resident (1/N folded into
  WQ on host).
- Phase B: per-edge Q gather replaced by a one-hot matmul against the SBUF
  Q-block (onehotT uploaded per subchunk from host as bf16 - no Q7 cost).
  KV gathers batched per 4096-edge window, 2 calls (A: src<32768 subchunks
  first, B rest) to amortize the ~1us SWDGE fixed cost.  Score replication
  runs on the Scalar engine; all other elementwise work on DVE.
"""

from contextlib import ExitStack

import numpy as np
import ml_dtypes

import concourse.bass as bass
import concourse.mybir as mybir
from concourse import bass_utils
from concourse.bacc import Bacc
from concourse.tile import TileContext

P = 128
N_NODES = 50000
N_EDGES = 1600000
IN_DIM = 128
NUM_HEADS = 8
OUT_DIM = 16
N_CORES = 8
NODES_PER_CORE = N_NODES // N_CORES  # 6250
I16_BASE = 32768
GROUP_J = 8          # subchunks per compute group (1024 edges)
WIN_G = 4            # groups per gather window (4096 edges)
WIN_J = GROUP_J * WIN_G

F32 = mybir.dt.float32
BF16 = mybir.dt.bfloat16
F8 = mybir.dt.float8e4
I16 = mybir.dt.int16

ACT_REP = True      # replicate score on Scalar engine (else DVE bcast mult)
GCAP = 8             # max subchunks per gather call (8 -> 1024 idxs, one
                     # 64-descriptor packet per SDMA engine; multi-packet
                     # mode degenerates to one packet per descriptor)
QSEL_SPLIT = False    # per-subchunk bank-aligned Qsel PSUM tiles


def _ceil_to(x, m):
    return ((x + m - 1) // m) * m


def _bf(a):
    return np.asarray(a, dtype=np.float32).astype(ml_dtypes.bfloat16)


def _bf_bits(a):
    """bf16 bit pattern as int16 (for fused i16 uploads, bitcast on device)."""
    return _bf(a).view(np.int16)


def shard_edges(src, dst):
    """Partition edges by dst range; per core sort by dst; per 128-dst-block
    split into A (src<32768) / B subchunks of 128 edges; shared schedule
    (max counts over cores); subchunks regrouped per 32-subchunk window with
    A-subchunks first so each window needs only 2 gather calls."""
    src = np.asarray(src).astype(np.int64)
    dst = np.asarray(dst).astype(np.int64)

    order = np.argsort(dst, kind="stable")
    ds, ss = dst[order], src[order]
    bounds = np.searchsorted(ds, np.arange(N_CORES + 1) * NODES_PER_CORE)
    n_blocks = _ceil_to(NODES_PER_CORE, P) // P  # 49

    # per (core, block) A/B edge lists
    edges = [[None] * n_blocks for _ in range(N_CORES)]
    nA = np.zeros((N_CORES, n_blocks), dtype=np.int64)
    nB = np.zeros((N_CORES, n_blocks), dtype=np.int64)
    for c in range(N_CORES):
        sl = slice(bounds[c], bounds[c + 1])
        loc = ds[sl] - c * NODES_PER_CORE
        sc = ss[sl]
        bs = np.searchsorted(loc // P, np.arange(n_blocks + 1))
        for b in range(n_blocks):
            s2 = slice(bs[b], bs[b + 1])
            l2, s3 = loc[s2], sc[s2]
            am = s3 < I16_BASE
            edges[c][b] = ((s3[am], l2[am]), (s3[~am], l2[~am]))
            nA[c, b] = int(am.sum())
            nB[c, b] = int((~am).sum())

    subA = ((nA.max(axis=0) + P - 1) // P).astype(np.int64)
    subB = ((nB.max(axis=0) + P - 1) // P).astype(np.int64)
    if (subA + subB).sum() == 0:
        subA[0] = 1

    # subchunk list in block order: (block, tag)
    subs = []
    for b in range(n_blocks):
        subs += [(b, 0)] * int(subA[b]) + [(b, 1)] * int(subB[b])
    pad = (-len(subs)) % WIN_J
    subs += [(n_blocks - 1, 0)] * pad          # dummy subchunks (rel=-1)
    S = len(subs)
    n_win = S // WIN_J

    # reorder within each window: A first, then B (stable keeps block order)
    perm = []
    for w in range(n_win):
        ws = list(range(w * WIN_J, (w + 1) * WIN_J))
        perm += [j for j in ws if subs[j][1] == 0] + [j for j in ws if subs[j][1] == 1]
    subs = [subs[j] for j in perm]
    blk_of = [s[0] for s in subs]
    tags = [s[1] for s in subs]
    nA_win = [sum(1 for j in range(w * WIN_J, (w + 1) * WIN_J) if tags[j] == 0)
              for w in range(n_win)]

    # real (non-dummy) subchunk positions per (block, tag), in order
    slot_of = {}       # (block, tag) -> list of subchunk indices
    for i, (b, t) in enumerate(subs):
        slot_of.setdefault((b, t), []).append(i)
    n_dummy_tail = pad

    # fill per-core data
    kvidx = np.zeros((N_CORES, S * P), dtype=np.int16)
    rel = np.full((N_CORES, S, P), -1.0, dtype=np.float32)
    for c in range(N_CORES):
        for b in range(n_blocks):
            for t in range(2):
                sa, la = edges[c][b][t]
                slots = slot_of.get((b, t), [])
                base = 0 if t == 0 else I16_BASE
                for k, sidx in enumerate(slots):
                    lo = k * P
                    hi = min(lo + P, len(sa))
                    if hi <= lo:
                        break
                    n = hi - lo
                    kvidx[c, sidx * P : sidx * P + n] = (sa[lo:hi] - base).astype(np.int16)
                    rel[c, sidx, :n] = la[lo:hi] - b * P

    # first/last OCCURRENCE per block (A/B reorder interleaves blocks, so a
    # block's subchunks are not contiguous; PSUM tiles keyed by block)
    first_seen, last_seen = {}, {}
    for i, b in enumerate(blk_of):
        if b not in first_seen:
            first_seen[b] = i
        last_seen[b] = i
    first_of = [first_seen[blk_of[i]] == i for i in range(S)]
    last_of = [last_seen[blk_of[i]] == i for i in range(S)]

    # gather idx layout: per window [128, WIN_J*8] int16, wrapped in 16
    # partitions (idx i at partition i%16, col i//16) replicated x8
    kvidx_w = kvidx.reshape(N_CORES, n_win, WIN_J * 8, 16).transpose(0, 1, 3, 2)
    kvidx_w = np.ascontiguousarray(np.tile(kvidx_w, (1, 1, 8, 1)))  # [C,W,128,WIN_J*8]

    # rel per subchunk in partition-edge layout [128, S] (edge on partition)
    rel_pe = rel.transpose(0, 2, 1)                       # [C, P, S]
    rel_bits = _bf_bits(rel_pe)                           # int16 view of bf16

    # fused per-window i16 upload: [128, WIN_J*8 idx | WIN_J rel]
    fused = np.zeros((N_CORES, n_win, P, WIN_J * 8 + WIN_J), dtype=np.int16)
    fused[:, :, :, : WIN_J * 8] = kvidx_w
    fused[:, :, :, WIN_J * 8 :] = rel_bits.reshape(N_CORES, P, n_win, WIN_J).transpose(0, 2, 1, 3)

    # onehotT upload (fp8: 0/1 exact), window-contiguous:
    # [n_win, 128(node), WIN_J*128] (ohT_w[w, p, j*128+n] = 1 iff
    # rel[subchunk w*W+j, edge n] == p)
    onehotT = np.zeros((N_CORES, S, P, P), dtype=ml_dtypes.float8_e4m3fn)
    r = rel.astype(np.int64)                               # [C, S, P]
    cc, ssi, ee = np.nonzero(r >= 0)
    onehotT[cc, ssi, r[cc, ssi, ee], ee] = 1.0
    onehotT = np.ascontiguousarray(
        onehotT.reshape(N_CORES, n_win, WIN_J, P, P).transpose(0, 1, 3, 2, 4)
    ).reshape(N_CORES, n_win, P, WIN_J * P)

    return {
        "fused": fused,
        "onehotT": onehotT,
        "tags": tags,
        "blk_of": blk_of,
        "first_of": first_of,
        "last_of": last_of,
        "nA_win": nA_win,
        "n_blocks": n_blocks,
        "S": S,
        "n_win": n_win,
    }


def build_program(*, n_nodes_pad, nodes_core_pad, sched):
    nkv_chunks = n_nodes_pad // P          # 391
    nq_chunks = nodes_core_pad // P        # 49
    n_win = sched["n_win"]
    blk_of = sched["blk_of"]
    first_of = sched["first_of"]
    last_of = sched["last_of"]
    nA_win = sched["nA_win"]
    J, W = GROUP_J, WIN_J

    nc = Bacc(num_swdge_queues=4)

    hT_d = nc.dram_tensor("hT_d", [P, n_nodes_pad], BF16, kind="ExternalInput")
    hqT_d = nc.dram_tensor("hqT_d", [P, nodes_core_pad], BF16, kind="ExternalInput")
    wkv_d = nc.dram_tensor("wkv_d", [P, 2 * IN_DIM], BF16, kind="ExternalInput")
    wq_d = nc.dram_tensor("wq_d", [P, IN_DIM], BF16, kind="ExternalInput")
    fused_d = nc.dram_tensor("fused_d", [n_win, P, W * 8 + W], I16, kind="ExternalInput")
    ohT_d = nc.dram_tensor(
        "ohT_d", [n_win, P, W * P], F8, kind="ExternalInput"
    )
    wv_out = nc.dram_tensor(
        "wv_out", [sched["n_blocks"] * P, IN_DIM], F32, kind="ExternalOutput"
    )

    iota_np = np.tile(np.arange(P, dtype=np.float32), (P, W))  # [P, W*P]
    iota_d = nc.inline_tensor(_bf(iota_np), name="iota_d")

    with TileContext(nc) as tc:
        with (
            tc.tile_pool(name="const", bufs=1) as constp,
            tc.tile_pool(name="dram", bufs=1, space="DRAM") as dramp,
        ):
            iota = constp.tile([P, W * P], BF16)
            nc.sync.dma_start(iota, iota_d[:])
            wkv = constp.tile([P, 2 * IN_DIM], BF16, name="wkv")
            nc.sync.dma_start(wkv, wkv_d[:])
            wq = constp.tile([P, IN_DIM], BF16, name="wq")
            nc.sync.dma_start(wq, wq_d[:])

            kv_tab = dramp.tile([n_nodes_pad, 2 * IN_DIM], BF16, name="kv_tab")
            q_tab = constp.tile([P, nq_chunks * IN_DIM], BF16, name="q_tab")

            # ---- Phase A: projections (pools freed before phase B) ----
            SLAB = 16  # chunks per hT slab load (16*128 cols = 0.5MB)
            with (
                tc.tile_pool(name="hslab", bufs=3) as hslabp,
                tc.tile_pool(name="proj_ps", bufs=4, space="PSUM") as projps,
                tc.tile_pool(name="kvstage", bufs=3) as kvstagep,
            ):
                for s0 in range(0, nkv_chunks, SLAB):
                    s1 = min(s0 + SLAB, nkv_chunks)
                    slab = hslabp.tile([P, SLAB * P], BF16, name="slab")
                    nc.sync.dma_start(slab[:, : (s1 - s0) * P], hT_d[:, s0 * P : s1 * P])
                    stage = kvstagep.tile([P, SLAB * 2 * IN_DIM], BF16, name="kvstage")
                    for ci in range(s0, s1):
                        k = ci - s0
                        ps = projps.tile([P, 2 * IN_DIM], F32, name="ps_kv")
                        nc.tensor.matmul(
                            ps, lhsT=slab[:, k * P : (k + 1) * P], rhs=wkv,
                            start=True, stop=True,
                        )
                        dst = stage[:, k * 2 * IN_DIM : (k + 1) * 2 * IN_DIM]
                        if ci % 2 == 0:
                            nc.scalar.copy(dst, ps)
                        else:
                            nc.vector.tensor_copy(dst, ps)
                    nc.sync.dma_start(
                        kv_tab[s0 * P : s1 * P, :].rearrange(
                            "(c p) f -> p c f", p=P
                        ),
                        stage.rearrange("p (c f) -> p c f", f=2 * IN_DIM)[:, : s1 - s0, :],
                    )
                # Q projections -> SBUF-resident q_tab
                hq_slab = hslabp.tile([P, nq_chunks * P], BF16, name="hq_slab")
                nc.sync.dma_start(hq_slab, hqT_d[:, : nq_chunks * P])
                for ci in range(nq_chunks):
                    ps = projps.tile([P, IN_DIM], F32, name="ps_q")
                    nc.tensor.matmul(
                        ps, lhsT=hq_slab[:, ci * P : (ci + 1) * P], rhs=wq,
                        start=True, stop=True,
                    )
                    if ci % 2 == 0:
                        nc.scalar.copy(q_tab[:, ci * IN_DIM : (ci + 1) * IN_DIM], ps)
                    else:
                        nc.vector.tensor_copy(q_tab[:, ci * IN_DIM : (ci + 1) * IN_DIM], ps)

            kv_lo = kv_tab[0:I16_BASE, :]
            kv_hi = kv_tab[I16_BASE:n_nodes_pad, :]

            # ---- Phase B ----
            gq = [0]  # rotating SWDGE queue counter
            stack = ExitStack()
            winp = stack.enter_context(tc.tile_pool(name="win", bufs=5))
            grpp = stack.enter_context(tc.tile_pool(name="grp", bufs=4))
            qselps = stack.enter_context(
                tc.tile_pool(name="qsel_ps", bufs=2, space="PSUM")
            )
            wvps = stack.enter_context(tc.tile_pool(name="wv_ps", bufs=3, space="PSUM"))
            outp = stack.enter_context(tc.tile_pool(name="outst", bufs=3))
            wv_tiles = {}
            for w in range(n_win):
                fused_t = winp.tile([P, W * 8 + W], I16, name="fused_t")
                nc.sync.dma_start(fused_t, fused_d[w])
                kv_rows = winp.tile([P, W * 2 * IN_DIM], BF16, name="kv_rows")
                nA = nA_win[w]
                runs = []
                if nA > 0:
                    runs.append((0, nA, 0))
                if nA < W:
                    runs.append((nA, W, 1))
                split_runs = []
                for (j0, j1, tag) in runs:
                    for jk in range(j0, j1, GCAP):
                        split_runs.append((jk, min(jk + GCAP, j1), tag))
                for (j0, j1, tag) in split_runs:
                    nidx = (j1 - j0) * P
                    nc.gpsimd.dma_gather(
                        out_ap=kv_rows[
                            :, j0 * 2 * IN_DIM : j1 * 2 * IN_DIM
                        ].rearrange("p (c f) -> p c f", f=2 * IN_DIM),
                        in_ap=kv_hi if tag else kv_lo,
                        idxs_ap=fused_t[:, j0 * 8 : j1 * 8],
                        num_idxs=nidx,
                        num_idxs_reg=nidx,
                        elem_size=2 * IN_DIM,
                        single_packet=(nidx <= 1024),
                        queue_num=gq[0] % 4,
                    )
                    gq[0] += 1
                rel_all = fused_t[:, W * 8 :].bitcast(BF16)  # [P, W] bf16

                # scatter one-hot [edge, (j, node)] for the whole window
                onehot_w = winp.tile([P, W * P], BF16, name="onehot_w")
                nc.vector.tensor_tensor(
                    out=onehot_w.rearrange("p (j n) -> p j n", j=W),
                    in0=iota.rearrange("p (j n) -> p j n", j=W),
                    in1=rel_all.unsqueeze(-1).to_broadcast([P, W, P]),
                    op=mybir.AluOpType.is_equal,
                )
                # Q selection one-hot (transposed, fp8): one window-sized load
                # on the scalar engine's HWDGE ring (parallel to the SP ring)
                ohT_w = winp.tile([P, W * P], F8, name="ohT_w")
                nc.scalar.dma_start(ohT_w, ohT_d[w])

                for g in range(WIN_G):
                    sc0 = w * W + g * J    # first subchunk index of group
                    kvg = kv_rows[:, g * J * 2 * IN_DIM : (g + 1) * J * 2 * IN_DIM]
                    kv3 = kvg.rearrange("p (j f) -> p j f", f=2 * IN_DIM)
                    k3 = kv3[:, :, 0:IN_DIM]
                    v3 = kv3[:, :, IN_DIM : 2 * IN_DIM]
                    onehot = onehot_w[:, g * J * P : (g + 1) * J * P]
                    ohT = ohT_w[:, g * J * P : (g + 1) * J * P]
                    # Qsel[e, f] per subchunk via PE, then kq = K * Qsel
                    kq = grpp.tile([P, J * IN_DIM], BF16, name="kq")
                    if QSEL_SPLIT:
                        for j in range(J):
                            b = blk_of[sc0 + j]
                            qsel_ps = qselps.tile([P, IN_DIM], F32, name="qsel_ps")
                            nc.tensor.matmul(
                                qsel_ps,
                                lhsT=ohT[:, j * P : (j + 1) * P],
                                rhs=q_tab[:, b * IN_DIM : (b + 1) * IN_DIM],
                                start=True, stop=True,
                            )
                            nc.vector.tensor_tensor(
                                out=kq[:, j * IN_DIM : (j + 1) * IN_DIM],
                                in0=k3[:, j, :],
                                in1=qsel_ps,
                                op=mybir.AluOpType.mult,
                            )
                    else:
                        qsel_ps = qselps.tile([P, J * IN_DIM], F32, name="qsel_ps")
                        for j in range(J):
                            b = blk_of[sc0 + j]
                            nc.tensor.matmul(
                                qsel_ps[:, j * IN_DIM : (j + 1) * IN_DIM],
                                lhsT=ohT[:, j * P : (j + 1) * P],
                                rhs=q_tab[:, b * IN_DIM : (b + 1) * IN_DIM],
                                start=True, stop=True,
                            )
                        qsel = grpp.tile([P, J * IN_DIM], BF16, name="qsel")
                        nc.scalar.copy(qsel, qsel_ps)
                        nc.vector.tensor_tensor(
                            out=kq.rearrange("p (j f) -> p j f", f=IN_DIM),
                            in0=k3,
                            in1=qsel.rearrange("p (j f) -> p j f", f=IN_DIM),
                            op=mybir.AluOpType.mult,
                        )
                    # score[e, (j,h)] = reduce_d kq (bf16 accum: 16-term dot,
                    # keeps all reduce APs 2-byte for the DVE 2x perf mode)
                    score = grpp.tile([P, J * NUM_HEADS], BF16, name="score")
                    with nc.allow_low_precision("16-term bf16 dot product"):
                        nc.vector.tensor_reduce(
                            out=score,
                            in_=kq.rearrange("p (jh d) -> p jh d", d=OUT_DIM),
                            axis=mybir.AxisListType.X,
                            op=mybir.AluOpType.add,
                        )
                    msg = grpp.tile([P, J * IN_DIM], BF16, name="msg")
                    if ACT_REP:
                        score_rep = grpp.tile([P, J * IN_DIM], BF16, name="score_rep")
                        nc.scalar.copy(
                            score_rep.rearrange("p (jh d) -> p jh d", d=OUT_DIM),
                            score.unsqueeze(-1).to_broadcast(
                                [P, J * NUM_HEADS, OUT_DIM]
                            ),
                        )
                        nc.vector.tensor_tensor(
                            out=msg.rearrange("p (j f) -> p j f", f=IN_DIM),
                            in0=v3,
                            in1=score_rep.rearrange("p (j f) -> p j f", f=IN_DIM),
                            op=mybir.AluOpType.mult,
                        )
                    else:
                        # single broadcast TT: at DVE 1x a stride-0 innermost
                        # operand costs nothing, and the ACT score_rep stage
                        # drops out of the dependency chain
                        nc.vector.tensor_tensor(
                            out=msg.rearrange(
                                "p (j h d) -> p j h d", h=NUM_HEADS, d=OUT_DIM
                            ),
                            in0=v3.rearrange(
                                "p j (h d) -> p j h d", d=OUT_DIM
                            ),
                            in1=score.rearrange("p (j h) -> p j h", h=NUM_HEADS)
                            .unsqueeze(-1)
                            .to_broadcast([P, J, NUM_HEADS, OUT_DIM]),
                            op=mybir.AluOpType.mult,
                        )
                    # scatter-accumulate into block tiles
                    for j in range(J):
                        sc = sc0 + j
                        b = blk_of[sc]
                        if first_of[sc]:
                            wv_tiles[b] = wvps.tile([P, IN_DIM], F32, name="wv_tile")
                        nc.tensor.matmul(
                            wv_tiles[b],
                            lhsT=onehot[:, j * P : (j + 1) * P],
                            rhs=msg[:, j * IN_DIM : (j + 1) * IN_DIM],
                            start=first_of[sc],
                            stop=last_of[sc],
                        )
                        if last_of[sc]:
                            stage = outp.tile([P, IN_DIM], F32, name="stage")
                            nc.scalar.copy(stage, wv_tiles.pop(b))
                            nc.sync.dma_start(wv_out[b * P : (b + 1) * P, :], stage)
            stack.close()

    nc.finalize()
    return nc


def _make_in_maps(h, h_add, WQ, WK, WV, sched, n_nodes_pad, nodes_core_pad):
    h = np.asarray(h, dtype=np.float32)
    h_add = np.asarray(h_add, dtype=np.float32)
    hT = np.zeros((P, n_nodes_pad), dtype=ml_dtypes.bfloat16)
    hT[:, :N_NODES] = _bf(h.T)
    wkv = np.concatenate(
        [np.asarray(WK, np.float32), np.asarray(WV, np.float32)], axis=1
    )
    wkv = _bf(wkv)
    wq = _bf(np.asarray(WQ, np.float32) / float(N_NODES))
    in_maps = []
    for c in range(N_CORES):
        hqT = np.zeros((P, nodes_core_pad), dtype=ml_dtypes.bfloat16)
        hqT[:, :NODES_PER_CORE] = _bf(
            h_add[c * NODES_PER_CORE : (c + 1) * NODES_PER_CORE].T
        )
        in_maps.append(
            {
                "hT_d": hT,
                "hqT_d": hqT,
                "wkv_d": wkv,
                "wq_d": wq,
                "fused_d": sched["fused"][c],
                "ohT_d": sched["onehotT"][c],
            }
        )
    return in_maps


_TRACE = {"trace": False, "last": None, "tmpdir": None}


def kernel(h, h_add, src, dst, WQ, WK, WV):
    sched = shard_edges(src, dst)
    n_nodes_pad = _ceil_to(N_NODES, P)
    nodes_core_pad = _ceil_to(NODES_PER_CORE, P)

    nc = build_program(
        n_nodes_pad=n_nodes_pad, nodes_core_pad=nodes_core_pad, sched=sched
    )
    in_maps = _make_in_maps(h, h_add, WQ, WK, WV, sched, n_nodes_pad, nodes_core_pad)

    res = bass_utils.run_bass_kernel_spmd(
        nc,
        in_maps,
        core_ids=list(range(N_CORES)),
        trace=_TRACE["trace"],
        tmpdir=_TRACE["tmpdir"],
    )
    _TRACE["last"] = res

    out = np.concatenate(
        [np.asarray(res.results[c]["wv_out"])[:NODES_PER_CORE] for c in range(N_CORES)],
        axis=0,
    )
    return out.reshape(N_NODES, NUM_HEADS, OUT_DIM).astype(np.float32)



# revision 43
# speedup vs baseline: 1.3533x; 1.0149x over previous
"""Trainium2 Bass kernel for CrossFormerAttention-style GNN message passing.

Reference (N=50000 nodes, E=1600000 edges, 8 heads x 16 dims):
    Qh = (h_add @ WQ).reshape(N, 8, 16)
    Kh = (h @ WK).reshape(N, 8, 16)
    Vh = (h @ WV).reshape(N, 8, 16)
    score = sum(Kh[src] * Qh[dst], -1)             # [E, 8, 1]
    wV = segment_sum(Vh[src] * score, dst, N)      # [N, 8, 16]
    out = wV / N

Sharding: edges partitioned by dst range across 8 cores (6250 nodes/core);
each core owns a disjoint output slice -> no collective.

v3 design (1.33ms baseline -> ~0.91ms):
- Phase A: host uploads h TRANSPOSED in bf16 ([128, n_pad]); per 128-chunk
  the chunk is the PE lhsT directly, rhs is the fused WK|WV [128, 256];
  PSUM->SBUF copies alternate ACT/DVE (pipelined: hslab/proj/kvstage pools
  at bufs 3/4/3).  Q table stays SBUF-resident (1/N folded into WQ).
- Phase B: per-edge Q gather replaced by a one-hot matmul against the SBUF
  Q-block.  The Qsel one-hot (node-major) is uploaded from host in fp8
  (exact for 0/1, halves its DMA bytes; PE accepts fp8 lhsT x bf16 rhs),
  batched per window on the Scalar engine's HWDGE ring.  The scatter
  one-hot (edge-major) is built on DVE via iota==rel once per window.
  KV gathers stay at 1024 idxs/call (single_packet: 64-descriptor packets;
  multi-packet mode degenerates to one packet per descriptor and inflates
  Q7 time ~40%).  Score reduce accumulates in bf16.  Score replication on
  ACT; kq/reduce/msg on DVE.  Deep window pipelining (win bufs=5, grp
  bufs=4) hides gather latency.
- Measured walls (per core): DVE ~710us (all elementwise at 1x, 1.2GHz,
  128 lanes; the 2x 16-bit perf mode never engages), SWDGE descriptor
  generation on Q7 ~700us (~2.9ns/edge), SDMA ~620us.  These three are
  balanced within ~15%, so shifting work between them is net-neutral;
  uploading the scatter one-hot (v5) or reading Qsel PSUM from DVE (v6)
  both regressed.
"""

from contextlib import ExitStack

import numpy as np
import ml_dtypes

import concourse.bass as bass
import concourse.mybir as mybir
from concourse import bass_utils
from concourse.bacc import Bacc
from concourse.tile import TileContext

P = 128
N_NODES = 50000
N_EDGES = 1600000
IN_DIM = 128
NUM_HEADS = 8
OUT_DIM = 16
N_CORES = 8
NODES_PER_CORE = N_NODES // N_CORES  # 6250
I16_BASE = 32768
GROUP_J = 8          # subchunks per compute group (1024 edges)
WIN_G = 4            # groups per gather window (4096 edges)
WIN_J = GROUP_J * WIN_G

F32 = mybir.dt.float32
BF16 = mybir.dt.bfloat16
F8 = mybir.dt.float8e4
I16 = mybir.dt.int16

ACT_REP = True      # replicate score on Scalar engine (else DVE bcast mult)
GCAP = 8             # max subchunks per gather call (8 -> 1024 idxs, one
                     # 64-descriptor packet per SDMA engine; multi-packet
                     # mode degenerates to one packet per descriptor)
QSEL_SPLIT = False    # per-subchunk bank-aligned Qsel PSUM tiles


def _ceil_to(x, m):
    return ((x + m - 1) // m) * m


def _bf(a):
    return np.asarray(a, dtype=np.float32).astype(ml_dtypes.bfloat16)


def _bf_bits(a):
    """bf16 bit pattern as int16 (for fused i16 uploads, bitcast on device)."""
    return _bf(a).view(np.int16)


def shard_edges(src, dst):
    """Partition edges by dst range; per core sort by dst; per 128-dst-block
    split into A (src<32768) / B subchunks of 128 edges; shared schedule
    (max counts over cores); subchunks regrouped per 32-subchunk window with
    A-subchunks first so each window needs only 2 gather calls."""
    src = np.asarray(src).astype(np.int64)
    dst = np.asarray(dst).astype(np.int64)

    order = np.argsort(dst, kind="stable")
    ds, ss = dst[order], src[order]
    bounds = np.searchsorted(ds, np.arange(N_CORES + 1) * NODES_PER_CORE)
    n_blocks = _ceil_to(NODES_PER_CORE, P) // P  # 49

    # per (core, block) A/B edge lists
    edges = [[None] * n_blocks for _ in range(N_CORES)]
    nA = np.zeros((N_CORES, n_blocks), dtype=np.int64)
    nB = np.zeros((N_CORES, n_blocks), dtype=np.int64)
    for c in range(N_CORES):
        sl = slice(bounds[c], bounds[c + 1])
        loc = ds[sl] - c * NODES_PER_CORE
        sc = ss[sl]
        bs = np.searchsorted(loc // P, np.arange(n_blocks + 1))
        for b in range(n_blocks):
            s2 = slice(bs[b], bs[b + 1])
            l2, s3 = loc[s2], sc[s2]
            am = s3 < I16_BASE
            edges[c][b] = ((s3[am], l2[am]), (s3[~am], l2[~am]))
            nA[c, b] = int(am.sum())
            nB[c, b] = int((~am).sum())

    subA = ((nA.max(axis=0) + P - 1) // P).astype(np.int64)
    subB = ((nB.max(axis=0) + P - 1) // P).astype(np.int64)
    if (subA + subB).sum() == 0:
        subA[0] = 1

    # subchunk list in block order: (block, tag)
    subs = []
    for b in range(n_blocks):
        subs += [(b, 0)] * int(subA[b]) + [(b, 1)] * int(subB[b])
    pad = (-len(subs)) % WIN_J
    subs += [(n_blocks - 1, 0)] * pad          # dummy subchunks (rel=-1)
    S = len(subs)
    n_win = S // WIN_J

    # reorder within each window: A first, then B (stable keeps block order)
    perm = []
    for w in range(n_win):
        ws = list(range(w * WIN_J, (w + 1) * WIN_J))
        perm += [j for j in ws if subs[j][1] == 0] + [j for j in ws if subs[j][1] == 1]
    subs = [subs[j] for j in perm]
    blk_of = [s[0] for s in subs]
    tags = [s[1] for s in subs]
    nA_win = [sum(1 for j in range(w * WIN_J, (w + 1) * WIN_J) if tags[j] == 0)
              for w in range(n_win)]

    # real (non-dummy) subchunk positions per (block, tag), in order
    slot_of = {}       # (block, tag) -> list of subchunk indices
    for i, (b, t) in enumerate(subs):
        slot_of.setdefault((b, t), []).append(i)
    n_dummy_tail = pad

    # fill per-core data
    kvidx = np.zeros((N_CORES, S * P), dtype=np.int16)
    rel = np.full((N_CORES, S, P), -1.0, dtype=np.float32)
    for c in range(N_CORES):
        for b in range(n_blocks):
            for t in range(2):
                sa, la = edges[c][b][t]
                slots = slot_of.get((b, t), [])
                base = 0 if t == 0 else I16_BASE
                for k, sidx in enumerate(slots):
                    lo = k * P
                    hi = min(lo + P, len(sa))
                    if hi <= lo:
                        break
                    n = hi - lo
                    kvidx[c, sidx * P : sidx * P + n] = (sa[lo:hi] - base).astype(np.int16)
                    rel[c, sidx, :n] = la[lo:hi] - b * P

    # first/last OCCURRENCE per block (A/B reorder interleaves blocks, so a
    # block's subchunks are not contiguous; PSUM tiles keyed by block)
    first_seen, last_seen = {}, {}
    for i, b in enumerate(blk_of):
        if b not in first_seen:
            first_seen[b] = i
        last_seen[b] = i
    first_of = [first_seen[blk_of[i]] == i for i in range(S)]
    last_of = [last_seen[blk_of[i]] == i for i in range(S)]

    # gather idx layout: per window [128, WIN_J*8] int16, wrapped in 16
    # partitions (idx i at partition i%16, col i//16) replicated x8
    kvidx_w = kvidx.reshape(N_CORES, n_win, WIN_J * 8, 16).transpose(0, 1, 3, 2)
    kvidx_w = np.ascontiguousarray(np.tile(kvidx_w, (1, 1, 8, 1)))  # [C,W,128,WIN_J*8]

    # rel per subchunk in partition-edge layout [128, S] (edge on partition)
    rel_pe = rel.transpose(0, 2, 1)                       # [C, P, S]
    rel_bits = _bf_bits(rel_pe)                           # int16 view of bf16

    # fused per-window i16 upload: [128, WIN_J*8 idx | WIN_J rel]
    fused = np.zeros((N_CORES, n_win, P, WIN_J * 8 + WIN_J), dtype=np.int16)
    fused[:, :, :, : WIN_J * 8] = kvidx_w
    fused[:, :, :, WIN_J * 8 :] = rel_bits.reshape(N_CORES, P, n_win, WIN_J).transpose(0, 2, 1, 3)

    # onehotT upload (fp8: 0/1 exact), window-contiguous:
    # [n_win, 128(node), WIN_J*128] (ohT_w[w, p, j*128+n] = 1 iff
    # rel[subchunk w*W+j, edge n] == p)
    onehotT = np.zeros((N_CORES, S, P, P), dtype=ml_dtypes.float8_e4m3fn)
    r = rel.astype(np.int64)                               # [C, S, P]
    cc, ssi, ee = np.nonzero(r >= 0)
    onehotT[cc, ssi, r[cc, ssi, ee], ee] = 1.0
    onehotT = np.ascontiguousarray(
        onehotT.reshape(N_CORES, n_win, WIN_J, P, P).transpose(0, 1, 3, 2, 4)
    ).reshape(N_CORES, n_win, P, WIN_J * P)

    return {
        "fused": fused,
        "onehotT": onehotT,
        "tags": tags,
        "blk_of": blk_of,
        "first_of": first_of,
        "last_of": last_of,
        "nA_win": nA_win,
        "n_blocks": n_blocks,
        "S": S,
        "n_win": n_win,
    }


def build_program(*, n_nodes_pad, nodes_core_pad, sched):
    nkv_chunks = n_nodes_pad // P          # 391
    nq_chunks = nodes_core_pad // P        # 49
    n_win = sched["n_win"]
    blk_of = sched["blk_of"]
    first_of = sched["first_of"]
    last_of = sched["last_of"]
    nA_win = sched["nA_win"]
    J, W = GROUP_J, WIN_J

    nc = Bacc(num_swdge_queues=4)

    hT_d = nc.dram_tensor("hT_d", [P, n_nodes_pad], BF16, kind="ExternalInput")
    hqT_d = nc.dram_tensor("hqT_d", [P, nodes_core_pad], BF16, kind="ExternalInput")
    wkv_d = nc.dram_tensor("wkv_d", [P, 2 * IN_DIM], BF16, kind="ExternalInput")
    wq_d = nc.dram_tensor("wq_d", [P, IN_DIM], BF16, kind="ExternalInput")
    fused_d = nc.dram_tensor("fused_d", [n_win, P, W * 8 + W], I16, kind="ExternalInput")
    ohT_d = nc.dram_tensor(
        "ohT_d", [n_win, P, W * P], F8, kind="ExternalInput"
    )
    wv_out = nc.dram_tensor(
        "wv_out", [sched["n_blocks"] * P, IN_DIM], F32, kind="ExternalOutput"
    )

    iota_np = np.tile(np.arange(P, dtype=np.float32), (P, W))  # [P, W*P]
    iota_d = nc.inline_tensor(_bf(iota_np), name="iota_d")

    with TileContext(nc) as tc:
        with (
            tc.tile_pool(name="const", bufs=1) as constp,
            tc.tile_pool(name="dram", bufs=1, space="DRAM") as dramp,
        ):
            iota = constp.tile([P, W * P], BF16)
            nc.sync.dma_start(iota, iota_d[:])
            wkv = constp.tile([P, 2 * IN_DIM], BF16, name="wkv")
            nc.sync.dma_start(wkv, wkv_d[:])
            wq = constp.tile([P, IN_DIM], BF16, name="wq")
            nc.sync.dma_start(wq, wq_d[:])

            kv_tab = dramp.tile([n_nodes_pad, 2 * IN_DIM], BF16, name="kv_tab")
            q_tab = constp.tile([P, nq_chunks * IN_DIM], BF16, name="q_tab")

            # ---- Phase A: projections (pools freed before phase B) ----
            SLAB = 32  # chunks per hT slab load (32*128 cols = 1MB)
            with (
                tc.tile_pool(name="hslab", bufs=3) as hslabp,
                tc.tile_pool(name="proj_ps", bufs=4, space="PSUM") as projps,
                tc.tile_pool(name="kvstage", bufs=3) as kvstagep,
            ):
                for s0 in range(0, nkv_chunks, SLAB):
                    s1 = min(s0 + SLAB, nkv_chunks)
                    slab = hslabp.tile([P, SLAB * P], BF16, name="slab")
                    nc.sync.dma_start(slab[:, : (s1 - s0) * P], hT_d[:, s0 * P : s1 * P])
                    stage = kvstagep.tile([P, SLAB * 2 * IN_DIM], BF16, name="kvstage")
                    for ci in range(s0, s1):
                        k = ci - s0
                        ps = projps.tile([P, 2 * IN_DIM], F32, name="ps_kv")
                        nc.tensor.matmul(
                            ps, lhsT=slab[:, k * P : (k + 1) * P], rhs=wkv,
                            start=True, stop=True,
                        )
                        dst = stage[:, k * 2 * IN_DIM : (k + 1) * 2 * IN_DIM]
                        if ci % 2 == 0:
                            nc.scalar.copy(dst, ps)
                        else:
                            nc.vector.tensor_copy(dst, ps)
                    nc.sync.dma_start(
                        kv_tab[s0 * P : s1 * P, :].rearrange(
                            "(c p) f -> p c f", p=P
                        ),
                        stage.rearrange("p (c f) -> p c f", f=2 * IN_DIM)[:, : s1 - s0, :],
                    )
                # Q projections -> SBUF-resident q_tab
                hq_slab = hslabp.tile([P, nq_chunks * P], BF16, name="hq_slab")
                nc.sync.dma_start(hq_slab, hqT_d[:, : nq_chunks * P])
                for ci in range(nq_chunks):
                    ps = projps.tile([P, IN_DIM], F32, name="ps_q")
                    nc.tensor.matmul(
                        ps, lhsT=hq_slab[:, ci * P : (ci + 1) * P], rhs=wq,
                        start=True, stop=True,
                    )
                    if ci % 2 == 0:
                        nc.scalar.copy(q_tab[:, ci * IN_DIM : (ci + 1) * IN_DIM], ps)
                    else:
                        nc.vector.tensor_copy(q_tab[:, ci * IN_DIM : (ci + 1) * IN_DIM], ps)

            kv_lo = kv_tab[0:I16_BASE, :]
            kv_hi = kv_tab[I16_BASE:n_nodes_pad, :]

            # ---- Phase B ----
            gq = [0]  # rotating SWDGE queue counter
            stack = ExitStack()
            winp = stack.enter_context(tc.tile_pool(name="win", bufs=5))
            grpp = stack.enter_context(tc.tile_pool(name="grp", bufs=4))
            qselps = stack.enter_context(
                tc.tile_pool(name="qsel_ps", bufs=2, space="PSUM")
            )
            wvps = stack.enter_context(tc.tile_pool(name="wv_ps", bufs=3, space="PSUM"))
            outp = stack.enter_context(tc.tile_pool(name="outst", bufs=3))
            wv_tiles = {}
            for w in range(n_win):
                fused_t = winp.tile([P, W * 8 + W], I16, name="fused_t")
                nc.sync.dma_start(fused_t, fused_d[w])
                kv_rows = winp.tile([P, W * 2 * IN_DIM], BF16, name="kv_rows")
                nA = nA_win[w]
                runs = []
                if nA > 0:
                    runs.append((0, nA, 0))
                if nA < W:
                    runs.append((nA, W, 1))
                split_runs = []
                for (j0, j1, tag) in runs:
                    for jk in range(j0, j1, GCAP):
                        split_runs.append((jk, min(jk + GCAP, j1), tag))
                for (j0, j1, tag) in split_runs:
                    nidx = (j1 - j0) * P
                    nc.gpsimd.dma_gather(
                        out_ap=kv_rows[
                            :, j0 * 2 * IN_DIM : j1 * 2 * IN_DIM
                        ].rearrange("p (c f) -> p c f", f=2 * IN_DIM),
                        in_ap=kv_hi if tag else kv_lo,
                        idxs_ap=fused_t[:, j0 * 8 : j1 * 8],
                        num_idxs=nidx,
                        num_idxs_reg=nidx,
                        elem_size=2 * IN_DIM,
                        single_packet=(nidx <= 1024),
                        queue_num=gq[0] % 4,
                    )
                    gq[0] += 1
                rel_all = fused_t[:, W * 8 :].bitcast(BF16)  # [P, W] bf16

                # scatter one-hot [edge, (j, node)] for the whole window
                onehot_w = winp.tile([P, W * P], BF16, name="onehot_w")
                nc.vector.tensor_tensor(
                    out=onehot_w.rearrange("p (j n) -> p j n", j=W),
                    in0=iota.rearrange("p (j n) -> p j n", j=W),
                    in1=rel_all.unsqueeze(-1).to_broadcast([P, W, P]),
                    op=mybir.AluOpType.is_equal,
                )
                # Q selection one-hot (transposed, fp8): one window-sized load
                # on the scalar engine's HWDGE ring (parallel to the SP ring)
                ohT_w = winp.tile([P, W * P], F8, name="ohT_w")
                nc.scalar.dma_start(ohT_w, ohT_d[w])

                for g in range(WIN_G):
                    sc0 = w * W + g * J    # first subchunk index of group
                    kvg = kv_rows[:, g * J * 2 * IN_DIM : (g + 1) * J * 2 * IN_DIM]
                    kv3 = kvg.rearrange("p (j f) -> p j f", f=2 * IN_DIM)
                    k3 = kv3[:, :, 0:IN_DIM]
                    v3 = kv3[:, :, IN_DIM : 2 * IN_DIM]
                    onehot = onehot_w[:, g * J * P : (g + 1) * J * P]
                    ohT = ohT_w[:, g * J * P : (g + 1) * J * P]
                    # Qsel[e, f] per subchunk via PE, then kq = K * Qsel
                    kq = grpp.tile([P, J * IN_DIM], BF16, name="kq")
                    if QSEL_SPLIT:
                        for j in range(J):
                            b = blk_of[sc0 + j]
                            qsel_ps = qselps.tile([P, IN_DIM], F32, name="qsel_ps")
                            nc.tensor.matmul(
                                qsel_ps,
                                lhsT=ohT[:, j * P : (j + 1) * P],
                                rhs=q_tab[:, b * IN_DIM : (b + 1) * IN_DIM],
                                start=True, stop=True,
                            )
                            nc.vector.tensor_tensor(
                                out=kq[:, j * IN_DIM : (j + 1) * IN_DIM],
                                in0=k3[:, j, :],
                                in1=qsel_ps,
                                op=mybir.AluOpType.mult,
                            )
                    else:
                        qsel_ps = qselps.tile([P, J * IN_DIM], F32, name="qsel_ps")
                        for j in range(J):
                            b = blk_of[sc0 + j]
                            nc.tensor.matmul(
                                qsel_ps[:, j * IN_DIM : (j + 1) * IN_DIM],
                                lhsT=ohT[:, j * P : (j + 1) * P],
                                rhs=q_tab[:, b * IN_DIM : (b + 1) * IN_DIM],
                                start=True, stop=True,
                            )
                        qsel = grpp.tile([P, J * IN_DIM], BF16, name="qsel")
                        nc.scalar.copy(qsel, qsel_ps)
                        nc.vector.tensor_tensor(
                            out=kq.rearrange("p (j f) -> p j f", f=IN_DIM),
                            in0=k3,
                            in1=qsel.rearrange("p (j f) -> p j f", f=IN_DIM),
                            op=mybir.AluOpType.mult,
                        )
                    # score[e, (j,h)] = reduce_d kq (bf16 accum: 16-term dot,
                    # keeps all reduce APs 2-byte for the DVE 2x perf mode)
                    score = grpp.tile([P, J * NUM_HEADS], BF16, name="score")
                    with nc.allow_low_precision("16-term bf16 dot product"):
                        nc.vector.tensor_reduce(
                            out=score,
                            in_=kq.rearrange("p (jh d) -> p jh d", d=OUT_DIM),
                            axis=mybir.AxisListType.X,
                            op=mybir.AluOpType.add,
                        )
                    msg = grpp.tile([P, J * IN_DIM], BF16, name="msg")
                    if ACT_REP:
                        score_rep = grpp.tile([P, J * IN_DIM], BF16, name="score_rep")
                        nc.scalar.copy(
                            score_rep.rearrange("p (jh d) -> p jh d", d=OUT_DIM),
                            score.unsqueeze(-1).to_broadcast(
                                [P, J * NUM_HEADS, OUT_DIM]
                            ),
                        )
                        nc.vector.tensor_tensor(
                            out=msg.rearrange("p (j f) -> p j f", f=IN_DIM),
                            in0=v3,
                            in1=score_rep.rearrange("p (j f) -> p j f", f=IN_DIM),
                            op=mybir.AluOpType.mult,
                        )
                    else:
                        # single broadcast TT: at DVE 1x a stride-0 innermost
                        # operand costs nothing, and the ACT score_rep stage
                        # drops out of the dependency chain
                        nc.vector.tensor_tensor(
                            out=msg.rearrange(
                                "p (j h d) -> p j h d", h=NUM_HEADS, d=OUT_DIM
                            ),
                            in0=v3.rearrange(
                                "p j (h d) -> p j h d", d=OUT_DIM
                            ),
                            in1=score.rearrange("p (j h) -> p j h", h=NUM_HEADS)
                            .unsqueeze(-1)
                            .to_broadcast([P, J, NUM_HEADS, OUT_DIM]),
                            op=mybir.AluOpType.mult,
                        )
                    # scatter-accumulate into block tiles
                    for j in range(J):
                        sc = sc0 + j
                        b = blk_of[sc]
                        if first_of[sc]:
                            wv_tiles[b] = wvps.tile([P, IN_DIM], F32, name="wv_tile")
                        nc.tensor.matmul(
                            wv_tiles[b],
                            lhsT=onehot[:, j * P : (j + 1) * P],
                            rhs=msg[:, j * IN_DIM : (j + 1) * IN_DIM],
                            start=first_of[sc],
                            stop=last_of[sc],
                        )
                        if last_of[sc]:
                            stage = outp.tile([P, IN_DIM], F32, name="stage")
                            nc.scalar.copy(stage, wv_tiles.pop(b))
                            nc.sync.dma_start(wv_out[b * P : (b + 1) * P, :], stage)
            stack.close()

    nc.finalize()
    return nc


def _make_in_maps(h, h_add, WQ, WK, WV, sched, n_nodes_pad, nodes_core_pad):
    h = np.asarray(h, dtype=np.float32)
    h_add = np.asarray(h_add, dtype=np.float32)
    hT = np.zeros((P, n_nodes_pad), dtype=ml_dtypes.bfloat16)
    hT[:, :N_NODES] = _bf(h.T)
    wkv = np.concatenate(
        [np.asarray(WK, np.float32), np.asarray(WV, np.float32)], axis=1
    )
    wkv = _bf(wkv)
    wq = _bf(np.asarray(WQ, np.float32) / float(N_NODES))
    in_maps = []
    for c in range(N_CORES):
        hqT = np.zeros((P, nodes_core_pad), dtype=ml_dtypes.bfloat16)
        hqT[:, :NODES_PER_CORE] = _bf(
            h_add[c * NODES_PER_CORE : (c + 1) * NODES_PER_CORE].T
        )
        in_maps.append(
            {
                "hT_d": hT,
                "hqT_d": hqT,
                "wkv_d": wkv,
                "wq_d": wq,
                "fused_d": sched["fused"][c],
                "ohT_d": sched["onehotT"][c],
            }
        )
    return in_maps


_TRACE = {"trace": False, "last": None, "tmpdir": None}


def kernel(h, h_add, src, dst, WQ, WK, WV):
    sched = shard_edges(src, dst)
    n_nodes_pad = _ceil_to(N_NODES, P)
    nodes_core_pad = _ceil_to(NODES_PER_CORE, P)

    nc = build_program(
        n_nodes_pad=n_nodes_pad, nodes_core_pad=nodes_core_pad, sched=sched
    )
    in_maps = _make_in_maps(h, h_add, WQ, WK, WV, sched, n_nodes_pad, nodes_core_pad)

    res = bass_utils.run_bass_kernel_spmd(
        nc,
        in_maps,
        core_ids=list(range(N_CORES)),
        trace=_TRACE["trace"],
        tmpdir=_TRACE["tmpdir"],
    )
    _TRACE["last"] = res

    out = np.concatenate(
        [np.asarray(res.results[c]["wv_out"])[:NODES_PER_CORE] for c in range(N_CORES)],
        axis=0,
    )
    return out.reshape(N_NODES, NUM_HEADS, OUT_DIM).astype(np.float32)

